# revision 27
# baseline (speedup 1.0000x reference)
"""KAN embeddings Bass kernel for Trainium2, 8-core feature-parallel.

out[b,i,d] = silu(x[b,i]) * base_w[i,d] + sum_g exp(-0.5(x[b,i]-grid[g])^2) * gp_w[i,g,d]

Sharding: each core owns NFS = NF/8 = 32 features for the full batch.
This minimizes replicated-weight HBM traffic.

The device computes the GP branch only (97.8% of the FLOPs); the rank-1
base branch silu(x) (x) base_w is added exactly on the host during the
bf16->f32 gather. Dropping the base row keeps the contraction at K=64,
which enables 2x row-packing of the PE array (the PE runs at 1.2 GHz /
K=4/8 on this system, so array utilization is the scarce resource).

Per-core pipeline (4 blocks of 4 feature-pairs x 2048 batch):
  - RBF features via exp(-0.5 x^2 + g*x - 0.5 g^2): K=4 f32r matmuls
    with a block-diagonal stationary ([1; grid] on rows 0-1 -> output
    partitions 0-63 for the even feature, rows 2-3 -> partitions 64-127
    for the odd feature), row-packed pairwise (rows 0-3 / 32-35), then
    one ACT exp per PSUM(128, 1024) with per-partition bias -0.5 g^2,
    output bf16 into fb128. Feature gen is interleaved into the mains
    loop at one tile per two chunks (own batch-half-1 tiles in chunks
    1-7, next block's batch-half-0 tiles in chunks 9-15), so the serial
    matmul->exp chain (pt bufs=1) hides behind main-matmul/evac work
    and its pt-buf wait (an exp >=2 chunks old) never stalls the
    tensor FIFO.
  - Main matmuls in bf16: feature-pair row-packed - even feature on PE
    rows 0-63, odd on rows 64-127, concurrent, each K=64, N=512 ->
    adjacent PSUM banks. PSUM f32 -> SBUF bf16 copies (FD=1024)
    alternate DVE:ACT at 8:7; 1 MiB bf16 output DMAs (8 KiB contiguous
    per partition) all on the SP HWDGE ring, which sustains ~363 GB/s
    (the HBM-per-core write ceiling - the steady-state pacer).
"""

import numpy as np

B, NF, G, D = 2048, 256, 64, 512
NCORES = 8
NFS = NF // NCORES        # 32 features per core
NBLK = 8                  # features per block
NPAIR = NBLK // 2         # 4 feature pairs per block
NBLOCKS = NFS // NBLK     # 4
CH = 128                  # batch rows per output chunk
NCH = B // CH             # 16
XCH = 4096                # x4 cols per staged tile
NU = NPAIR * B // 1024    # 8 featgen psum tiles per block

_cache = {}


def _build():
    import concourse.bass as bass
    from concourse import mybir
    from concourse import tile

    f32 = mybir.dt.float32
    f32r = mybir.dt.float32r
    bf16 = mybir.dt.bfloat16
    AF = mybir.ActivationFunctionType

    nc = bass.Bass()
    x4 = nc.declare_dram_parameter("x4", [4, NFS // 2 * B], f32r, isOutput=False)
    wcat = nc.declare_dram_parameter(
        "wcat", [2 * G, NFS // 2, D], bf16, isOutput=False
    )
    s4 = nc.declare_dram_parameter("s4", [4, 2 * G], f32r, isOutput=False)
    nb2 = nc.declare_dram_parameter("nb2", [2 * G, 1], f32, isOutput=False)
    out = nc.declare_dram_parameter("out", [B, NFS, D], bf16, isOutput=True)

    state = {"ncopy": 0}

    with tile.TileContext(nc) as tc:
        with (
            tc.tile_pool(name="const", bufs=1) as constp,
            tc.tile_pool(name="x4p", bufs=4) as x4p,
            tc.tile_pool(name="fbp", bufs=2) as fbp,
            tc.tile_pool(name="wp", bufs=2) as wp,
            tc.tile_pool(name="stage", bufs=3) as stagep,
            tc.tile_pool(name="pt", bufs=1, space="PSUM") as ptp,
            tc.tile_pool(name="po", bufs=3, space="PSUM") as pop,
        ):
            # block-diagonal K=4 stationary: rows 0-1 map [1; grid] to
            # output cols 0-63 (even feature), rows 2-3 to cols 64-127
            # (odd feature); copy at rows 32-35 for featgen row-packing
            s4_t = constp.tile([36, 2 * G], f32r)
            nc.gpsimd.dma_start(out=s4_t[0:4, :], in_=s4[:, :])
            nc.gpsimd.dma_start(out=s4_t[32:36, :], in_=s4[:, :])
            nb2_t = constp.tile([2 * G, 1], f32)
            nc.gpsimd.dma_start(out=nb2_t[:, :], in_=nb2[:, :])

            def emit_block_loads(blk):
                """DMA x4 (rows duplicated at 32-35 for row-packing), the
                fb tile, and weights for block blk."""
                base = blk * NPAIR * B
                fb = fbp.tile([2 * G, NPAIR * B], bf16)
                x4_tiles = []
                for h in range(NPAIR * B // XCH):  # 2
                    xt = x4p.tile([36, XCH], f32r)
                    lo = base + h * XCH
                    nc.gpsimd.dma_start(out=xt[0:4, :], in_=x4[0:4, lo:lo + XCH])
                    nc.gpsimd.dma_start(out=xt[32:36, :], in_=x4[0:4, lo:lo + XCH])
                    x4_tiles.append(xt)
                wt = wp.tile([2 * G, NPAIR * D], bf16)
                nc.sync.dma_start(
                    out=wt[:, :].rearrange("g (q d) -> g q d", q=NPAIR),
                    in_=wcat[:, blk * NPAIR:(blk + 1) * NPAIR, :],
                )
                return x4_tiles, fb, wt

            def emit_featgen_tile(x4_tiles, fb, u):
                """1024 pair-cols: two row-packed K=4 block-diagonal
                matmuls (rows 0-3 / 32-35, concurrent) + one exp."""
                pt = ptp.tile([2 * G, 1024], f32)
                for cb in range(2):
                    h, off = divmod(u * 1024 + cb * 512, XCH)
                    r0 = 32 * cb
                    nc.tensor.matmul(
                        pt[:, cb * 512:(cb + 1) * 512],
                        s4_t[r0:r0 + 4, :],
                        x4_tiles[h][r0:r0 + 4, off:off + 512],
                        start=True,
                        stop=True,
                    )
                nc.scalar.activation(
                    fb[:, u * 1024:(u + 1) * 1024],
                    pt[:, :],
                    AF.Exp,
                    bias=nb2_t[:, :],
                    scale=1.0,
                )

            def emit_mains_chunk(blk, fb, wt, c):
                """8 row-packed feature-pair matmul groups + copies + DMA.

                Whole-chunk engine assignment (alternating, 8 DVE : 7 ACT):
                all four PSUM->SBUF copies of a chunk run on one engine, and
                the chunk's output DMA rides that engine's HWDGE ring. An
                ACT-ring DMA then only waits on same-engine copies (already
                retired in FIFO order), so it never head-of-line blocks the
                ACT queue, and the two rings stream output concurrently."""
                gc = blk * NCH + c
                on_dve = gc % 15 % 2 == 0
                st = stagep.tile([CH, NBLK * D], bf16, tag="stage")
                for q in range(NPAIR):
                    po = pop.tile([CH, 1024], f32)
                    b0 = q * B + c * CH
                    for half in range(2):
                        nc.tensor.matmul(
                            po[:, half * 512:(half + 1) * 512],
                            fb[half * G:(half + 1) * G, b0:b0 + CH],
                            wt[half * G:(half + 1) * G, q * D:(q + 1) * D],
                            start=True,
                            stop=True,
                        )
                    dst = st[:, q * 1024:(q + 1) * 1024]
                    if on_dve:
                        nc.vector.tensor_copy(dst, po[:, :])
                    else:
                        nc.scalar.copy(dst, po[:, :])
                (nc.sync if on_dve else nc.scalar).dma_start(
                    out=out[c * CH:(c + 1) * CH,
                            blk * NBLK:(blk + 1) * NBLK, :],
                    in_=st[:, :].rearrange("b (i d) -> b i d", i=NBLK),
                )

            # prologue: block 0 loads + the 4 featgen tiles chunk 0-7
            # consume (even u = batch half 0)
            cur = emit_block_loads(0)
            for u in range(0, NU, 2):
                emit_featgen_tile(cur[0], cur[1], u)
            for blk in range(NBLOCKS):
                x4_tiles, fb, wt = cur
                nxt = emit_block_loads(blk + 1) if blk + 1 < NBLOCKS else None
                for c in range(NCH):
                    emit_mains_chunk(blk, fb, wt, c)
                    # featgen at half rate in odd-chunk slots: c=1..7 emit
                    # this block's odd tiles (needed from chunk 8), c=9..15
                    # emit the next block's even tiles. The pt-buf wait
                    # (an exp >=2 chunks old) never blocks the FIFO.
                    if c % 2 == 1:
                        if c < NCH // 2:
                            emit_featgen_tile(x4_tiles, fb, c)
                        elif nxt is not None:
                            emit_featgen_tile(nxt[0], nxt[1], c - NCH // 2 - 1)
                cur = nxt

    _split_multi_waits(nc)
    return nc


def _split_multi_waits(nc):
    """Walrus TPB instruction structs accept a single sync wait. Hoist all
    but the last wait of any instruction onto same-engine NOPs inserted
    immediately before it (a wait executes before the instruction either
    way, so this is semantically identical)."""
    import dataclasses
    import concourse.bass as bass
    import concourse.mybir as mybir

    tpl = bass.Bass().sync.nop().ins
    k = 0
    for blk in nc.m.functions[0].blocks:
        out_insts = []
        for inst in blk.instructions:
            si = getattr(inst, "sync_info", None)
            if si is not None and len(si.on_wait) > 1:
                for w in si.on_wait[:-1]:
                    out_insts.append(
                        dataclasses.replace(
                            tpl,
                            name=f"nop-w{k}",
                            engine=inst.engine,
                            sync_info=mybir.SyncInfo(on_wait=[w], on_update=[]),
                        )
                    )
                    k += 1
                inst.sync_info = dataclasses.replace(si, on_wait=si.on_wait[-1:])
            out_insts.append(inst)
        blk.instructions[:] = out_insts


def _prep_inputs(x, base_weight, gp_weight, grid):
    import ml_dtypes

    bf16 = ml_dtypes.bfloat16
    x = np.ascontiguousarray(np.asarray(x, np.float32))
    gp_weight = np.asarray(gp_weight, np.float32)
    grid = np.asarray(grid, np.float32)

    # (2G, NF/2, D) bf16: [g, q, d] = gp_w[2q + (g>=64), g%64, d] for the
    # row-packed pair layout (per-core feature pairs are local)
    gw = gp_weight.astype(bf16)  # (NF, G, D)
    s4 = np.zeros((4, 2 * G), np.float32)
    s4[0, 0:G] = 1.0
    s4[1, 0:G] = grid
    s4[2, G:2 * G] = 1.0
    s4[3, G:2 * G] = grid
    nb2 = np.ascontiguousarray(
        np.tile((-0.5 * grid * grid), 2).reshape(2 * G, 1)
    )

    in_maps = []
    for c in range(NCORES):
        i0 = c * NFS
        xT = np.ascontiguousarray(x[:, i0:i0 + NFS].T)  # (NFS, B)
        xe, xo = xT[0::2], xT[1::2]  # (NFS/2, B) even/odd features
        x4 = np.empty((4, NFS // 2 * B), np.float32)
        x4[0] = (-0.5 * xe * xe).ravel()
        x4[1] = xe.ravel()
        x4[2] = (-0.5 * xo * xo).ravel()
        x4[3] = xo.ravel()
        # wcat[g, q, d]: g<64 -> even feature of pair q, g>=64 -> odd
        wc = np.empty((2 * G, NFS // 2, D), bf16)
        wc[0:G] = gw[i0:i0 + NFS:2].transpose(1, 0, 2)
        wc[G:2 * G] = gw[i0 + 1:i0 + NFS:2].transpose(1, 0, 2)
        in_maps.append({
            "x4": x4,
            "wcat": np.ascontiguousarray(wc),
            "s4": s4,
            "nb2": nb2,
        })
    return in_maps


def _run(in_maps, **kw):
    from concourse.bass_utils import run_bass_kernel_spmd

    if "nc" not in _cache:
        _cache["nc"] = _build()
    return run_bass_kernel_spmd(_cache["nc"], in_maps, list(range(NCORES)), **kw)


def _gather(res, x, base_weight):
    """bf16 GP-branch shards -> f32 full output, plus the exact rank-1
    base branch silu(x) (x) base_w added on the host."""
    x = np.asarray(x, np.float32)
    bw = np.asarray(base_weight, np.float32)
    silu = x / (1.0 + np.exp(-x))  # (B, NF)
    full = np.empty((B, NF, D), np.float32)
    for c in range(NCORES):
        i0 = c * NFS
        shard = np.asarray(res.results[c]["out"]).astype(np.float32)
        shard += silu[:, i0:i0 + NFS, None] * bw[None, i0:i0 + NFS, :]
        full[:, i0:i0 + NFS, :] = shard
    return full


def kernel(x, base_weight, gp_weight, grid):
    in_maps = _prep_inputs(x, base_weight, gp_weight, grid)
    res = _run(in_maps)
    return _gather(res, x, base_weight)
